# revision 1
# baseline (speedup 1.0000x reference)
"""DCN-v1 (dense_mlp) Trainium2 kernel.

Strategy (8 NeuronCores, SPMD):
  - Data-parallel over batch: 16384 rows -> 2048 per core.
  - Embedding tables replicated per core (bf16, flattened per-field with
    host-side index offsetting); lookups via indirect DMA gathers.
  - Multi-hot sum-pool via one strided DVE reduction per 128-row chunk.
  - Activations kept feature-major (x^T) on chip; weights are the matmul
    stationary operand, batch streams as the moving operand.
  - CrossNet collapsed algebraically: x_i = A_i * x0 + C_i with A_i a
    per-sample scalar and C_i an input-derived constant vector, so the whole
    cross stack + final-layer cross dot reduce to ONE matmul
    P = x0 @ [w_0..w_3, lin_w[:448]] plus a tiny scalar recurrence.
  - MLP in bf16 with fp32 PSUM accumulation; ReLU+bias fused on ScalarE.

Performance notes (measured, 8 cores):
  - HW exec ~2.04 ms, rel err (l2) 1.15e-3 vs the fp32 reference.
  - The time is 1408 indirect-DMA gathers x ~1.45 us of serialized SWDGE
    descriptor generation; every other engine (PE ~75 us/core, DVE, ACT)
    and the transferred bytes hide completely under it.
  - The gather count is the floor for this primitive: indirect DMA moves
    exactly one table row per partition per instruction (multi-index offset
    APs silently collapse to idx[p,0] on HW, verified empirically).
  - dma_gather (InstDMAGatherAnt) would cut this ~6x via 4-row-grouped
    int16 super-indices + mask-fused sub-row select (see kernel2.py), but
    it does not execute under the current PJRT/axon runtime (leaves the
    exec unit unrecoverable; reproduced with a minimal standalone kernel).
"""

import os
import sys

import numpy as np
import ml_dtypes

for _p in ("/opt/trn_rl_repo", os.path.expanduser("~/.axon_site/_ro/trn_rl_repo")):
    if os.path.isdir(_p) and _p not in sys.path:
        sys.path.append(_p)

B = 16384
N_CORES = 8
BL = B // N_CORES  # 2048 rows per core
DENSE = 64
N_OH, N_MH, HIST = 8, 4, 20
VOCAB = 100000
EMB = 32
IN_DIM = 448
HID = [1024, 512, 256]
CHUNK = 128  # samples per gather/transpose chunk
NBLK = 512  # samples per matmul n-block
KS = [128, 128, 128, 64]  # k-tile sizes over the 448-dim input features
BF16 = ml_dtypes.bfloat16


def _build_program(c_consts, sig_bias, debug=False):
    """Build the SPMD Bass/Tile program. c_consts[l] = C_l . w_l (fp32 floats),
    sig_bias = C_4 . lin_w[:448] + lin_b."""
    from contextlib import ExitStack

    import concourse.bass as bass
    import concourse.tile as tile
    from concourse import bacc, mybir
    from concourse.masks import make_identity

    dt = mybir.dt
    AF = mybir.ActivationFunctionType
    n_chunks_per_nb = NBLK // CHUNK  # 4
    n_nb = BL // NBLK  # 4

    nc = bacc.Bacc()
    dense_d = nc.dram_tensor(
        "dense", [128, BL // 128, DENSE], dt.bfloat16, kind="ExternalInput"
    )
    idx_d = nc.dram_tensor("idx", [128, BL // 128, 88], dt.int32, kind="ExternalInput")
    ohtab_d = nc.dram_tensor(
        "ohtab", [N_OH * VOCAB, EMB], dt.bfloat16, kind="ExternalInput"
    )
    mhtab_d = nc.dram_tensor(
        "mhtab", [N_MH * VOCAB, EMB], dt.bfloat16, kind="ExternalInput"
    )
    w1_d = nc.dram_tensor("w1p", [128, 4, 1024], dt.bfloat16, kind="ExternalInput")
    w2_d = nc.dram_tensor("w2p", [128, 8, 512], dt.bfloat16, kind="ExternalInput")
    w3_d = nc.dram_tensor("w3p", [128, 4, 256], dt.bfloat16, kind="ExternalInput")
    wsm_d = nc.dram_tensor("wsm", [128, 22], dt.bfloat16, kind="ExternalInput")
    bias_d = nc.dram_tensor("biasp", [128, 14], dt.float32, kind="ExternalInput")
    out_d = nc.dram_tensor("out", [128, BL // 128], dt.float32, kind="ExternalOutput")
    if debug:
        dbgx_d = nc.dram_tensor(
            "dbg_x0", [128, BL // 128, 512], dt.bfloat16, kind="ExternalOutput"
        )
        dbgp_d = nc.dram_tensor(
            "dbg_pn", [128, BL // 128, 5], dt.float32, kind="ExternalOutput"
        )

    with ExitStack() as ctx:
        tc = ctx.enter_context(tile.TileContext(nc))
        wp = ctx.enter_context(tc.tile_pool(name="weights", bufs=1))
        # DMA-written tiles get one slot per chunk: the DIRECT2D pseudo-DMA
        # ISA struct only has a single sync-wait slot, so gathers must not
        # carry WAR/WAW waits from slot reuse.
        x0p = ctx.enter_context(tc.tile_pool(name="x0", bufs=3))
        gtp = ctx.enter_context(tc.tile_pool(name="gt", bufs=16))
        mhp = ctx.enter_context(tc.tile_pool(name="mh", bufs=2))
        xtp = ctx.enter_context(tc.tile_pool(name="xt", bufs=2))
        hp = ctx.enter_context(tc.tile_pool(name="h", bufs=2))
        recp = ctx.enter_context(tc.tile_pool(name="rec", bufs=2))
        ps_mm = ctx.enter_context(tc.tile_pool(name="psmm", bufs=3, space="PSUM"))
        ps_tr = ctx.enter_context(tc.tile_pool(name="pstr", bufs=2, space="PSUM"))
        ps_sm = ctx.enter_context(tc.tile_pool(name="pssm", bufs=1, space="PSUM"))
        ps_q2 = ctx.enter_context(tc.tile_pool(name="psq2", bufs=2, space="PSUM"))

        # --- resident weights ---
        w1_sb = wp.tile([128, 4, 1024], dt.bfloat16)
        nc.sync.dma_start(w1_sb[:], w1_d[:])
        w2_sb = wp.tile([128, 8, 512], dt.bfloat16)
        nc.sync.dma_start(w2_sb[:], w2_d[:])
        w3_sb = wp.tile([128, 4, 256], dt.bfloat16)
        nc.sync.dma_start(w3_sb[:], w3_d[:])
        wsm_sb = wp.tile([128, 22], dt.bfloat16)
        nc.sync.dma_start(wsm_sb[:], wsm_d[:])
        bias_sb = wp.tile([128, 14], dt.float32)
        nc.sync.dma_start(bias_sb[:], bias_d[:])
        ident = wp.tile([128, 128], dt.bfloat16)
        make_identity(nc, ident[:])
        # whole-core index + dense staging: one DMA each, resident in SBUF,
        # so per-chunk gathers carry no DMA-RAW waits (ISA wait-slot limits)
        idx_sb = wp.tile([128, BL // 128, 88], dt.int32)
        nc.sync.dma_start(idx_sb[:], idx_d[:])
        dense_sb = wp.tile([128, BL // 128, DENSE], dt.bfloat16)
        nc.sync.dma_start(dense_sb[:], dense_d[:])
        out_sb = wp.tile([128, BL // 128], dt.float32)

        for nb in range(n_nb):
            # ---- build x0^T [feat, 512] for this n-block, 128 samples at a time
            x0T = xtp.tile([128, 4, NBLK], dt.bfloat16, tag="x0T")
            lgq1 = recp.tile([128, 4], dt.float32, tag="lgq1")
            for cc in range(n_chunks_per_nb):
                c = nb * n_chunks_per_nb + cc
                rs = slice(c * CHUNK, (c + 1) * CHUNK)
                cs = slice(cc * CHUNK, (cc + 1) * CHUNK)

                # indirect DMA moves one table row per partition per
                # instruction (multi-index offset APs silently collapse to
                # idx[p,0] on HW), so each of the 88 lookup slots is its own
                # gather instruction.
                x0n = gtp.tile([128, 384], dt.bfloat16, tag="x0n")
                for k in range(N_OH):
                    nc.gpsimd.indirect_dma_start(
                        out=x0n[:, k * EMB : (k + 1) * EMB],
                        out_offset=None,
                        in_=ohtab_d[:, :],
                        in_offset=bass.IndirectOffsetOnAxis(
                            ap=idx_sb[:, c, k : k + 1], axis=0
                        ),
                    )
                mh_raw = gtp.tile([128, N_MH * HIST * EMB], dt.bfloat16, tag="mhraw")
                for k in range(N_MH * HIST):
                    nc.gpsimd.indirect_dma_start(
                        out=mh_raw[:, k * EMB : (k + 1) * EMB],
                        out_offset=None,
                        in_=mhtab_d[:, :],
                        in_offset=bass.IndirectOffsetOnAxis(
                            ap=idx_sb[:, c, 8 + k : 9 + k], axis=0
                        ),
                    )
                # sum-pool the 20-long history per field: strided reduce
                mh_ps = mhp.tile([128, N_MH * EMB], dt.float32, tag="mhpool")
                nc.vector.tensor_reduce(
                    out=mh_ps[:].rearrange("p (f e) -> p f e", f=N_MH),
                    in_=mh_raw[:].rearrange("p (f h e) -> p f e h", f=N_MH, h=HIST),
                    axis=mybir.AxisListType.X,
                    op=mybir.AluOpType.add,
                )
                nc.vector.tensor_copy(x0n[:, 256:384], mh_ps[:])

                # consolidate to a single-engine-writer tile: LDWEIGHTS (the
                # transpose reads x0 as the stationary operand) only supports
                # one sync wait, but the pieces come from several engines.
                x0c = x0p.tile([128, 512], dt.bfloat16, tag="x0c")
                nc.vector.memset(x0c[:, 448:512], 0.0)
                nc.vector.tensor_copy(x0c[:, 0:DENSE], dense_sb[:, c, :])
                nc.vector.tensor_copy(x0c[:, DENSE:448], x0n[:])

                # transpose the 128-sample chunk to feature-major
                tp = ps_tr.tile([128, 4, 128], dt.bfloat16, tag="trps")
                for j in range(4):
                    nc.tensor.transpose(
                        tp[:, j : j + 1, :],
                        x0c[:, j * 128 : (j + 1) * 128],
                        ident[:],
                    )
                nc.vector.tensor_copy(x0T[:, :, cs], tp[:])

                # cross projections for this chunk, sample-major:
                # pn[s, l] = x0 . w_l (l<4), pn[s, 4] = x0 . lin_w[:448]
                pn = ps_sm.tile([128, 5], dt.float32, tag="pn")
                for j in range(4):
                    nc.tensor.matmul(
                        pn[:],
                        x0T[0 : KS[j], j : j + 1, cs],
                        wsm_sb[0 : KS[j], j * 5 : j * 5 + 5],
                        start=(j == 0),
                        stop=(j == 3),
                    )
                # logit cross part: prod(1+p_l) * q1  (cross_b == 0)
                if debug:
                    nc.sync.dma_start(dbgx_d[:, c, :], x0c[:])
                    dbgp = recp.tile([128, 5], dt.float32, tag="dbgp")
                    nc.vector.tensor_copy(dbgp[:], pn[:])
                    nc.sync.dma_start(dbgp_d[:, c, :], dbgp[:])
                pp1 = recp.tile([128, 4], dt.float32, tag="pp1")
                nc.vector.tensor_scalar_add(pp1[:], pn[:, 0:4], 1.0)
                m01 = recp.tile([128, 1], dt.float32, tag="m01")
                nc.vector.tensor_mul(m01[:], pp1[:, 0:1], pp1[:, 1:2])
                m23 = recp.tile([128, 1], dt.float32, tag="m23")
                nc.vector.tensor_mul(m23[:], pp1[:, 2:3], pp1[:, 3:4])
                a4 = recp.tile([128, 1], dt.float32, tag="a4")
                nc.vector.tensor_mul(a4[:], m01[:], m23[:])
                nc.vector.tensor_mul(lgq1[:, cc : cc + 1], a4[:], pn[:, 4:5])

            # ---- deep net ----
            h1 = hp.tile([128, 8, NBLK], dt.bfloat16, tag="h1")
            for m in range(8):
                ps = ps_mm.tile([128, NBLK], dt.float32, tag="mm")
                for j in range(4):
                    nc.tensor.matmul(
                        ps[:],
                        w1_sb[0 : KS[j], j : j + 1, m * 128 : (m + 1) * 128],
                        x0T[0 : KS[j], j : j + 1, :],
                        start=(j == 0),
                        stop=(j == 3),
                    )
                nc.scalar.activation(
                    h1[:, m : m + 1, :], ps[:], AF.Relu, bias=bias_sb[:, m : m + 1]
                )
            h2 = hp.tile([128, 4, NBLK], dt.bfloat16, tag="h2")
            for m in range(4):
                ps = ps_mm.tile([128, NBLK], dt.float32, tag="mm")
                for j in range(8):
                    nc.tensor.matmul(
                        ps[:],
                        w2_sb[:, j : j + 1, m * 128 : (m + 1) * 128],
                        h1[:, j : j + 1, :],
                        start=(j == 0),
                        stop=(j == 7),
                    )
                nc.scalar.activation(
                    h2[:, m : m + 1, :], ps[:], AF.Relu, bias=bias_sb[:, 8 + m : 9 + m]
                )
            h3 = hp.tile([128, 2, NBLK], dt.bfloat16, tag="h3")
            for m in range(2):
                ps = ps_mm.tile([128, NBLK], dt.float32, tag="mm")
                for j in range(4):
                    nc.tensor.matmul(
                        ps[:],
                        w3_sb[:, j : j + 1, m * 128 : (m + 1) * 128],
                        h2[:, j : j + 1, :],
                        start=(j == 0),
                        stop=(j == 3),
                    )
                nc.scalar.activation(
                    h3[:, m : m + 1, :], ps[:], AF.Relu, bias=bias_sb[:, 12 + m : 13 + m]
                )

            # ---- final: logit = prod(1+p)*q1 + h3.lin_w_bot + sig_bias ----
            for cc in range(n_chunks_per_nb):
                c = nb * n_chunks_per_nb + cc
                cs = slice(cc * CHUNK, (cc + 1) * CHUNK)
                q2n = ps_q2.tile([128, 1], dt.float32, tag="q2n")
                for j in range(2):
                    nc.tensor.matmul(
                        q2n[:],
                        h3[:, j : j + 1, cs],
                        wsm_sb[:, 20 + j : 21 + j],
                        start=(j == 0),
                        stop=(j == 1),
                    )
                lg2 = recp.tile([128, 1], dt.float32, tag="lg2")
                nc.vector.tensor_add(lg2[:], lgq1[:, cc : cc + 1], q2n[:])
                nc.scalar.activation(
                    out_sb[:, c : c + 1], lg2[:], AF.Sigmoid, bias=float(sig_bias)
                )

        nc.sync.dma_start(out_d[:], out_sb[:])

    nc.compile()
    return nc


def _prep_inputs(
    dense_x,
    one_hot_x,
    multi_hot_x,
    one_hot_emb,
    multi_hot_emb,
    cross_w,
    cross_b,
    W1,
    b1,
    W2,
    b2,
    W3,
    b3,
    lin_w,
    lin_b,
):
    dense_bf = np.ascontiguousarray(dense_x, dtype=np.float32).astype(BF16)
    oh_tab = np.ascontiguousarray(
        one_hot_emb.reshape(N_OH * VOCAB, EMB), dtype=np.float32
    ).astype(BF16)
    mh_tab = np.ascontiguousarray(
        multi_hot_emb.reshape(N_MH * VOCAB, EMB), dtype=np.float32
    ).astype(BF16)

    oh_idx = one_hot_x.astype(np.int64) + (np.arange(N_OH, dtype=np.int64) * VOCAB)
    mh_idx = multi_hot_x.astype(np.int64) + (
        np.arange(N_MH, dtype=np.int64) * VOCAB
    ).reshape(1, N_MH, 1)
    idx_all = np.concatenate(
        [oh_idx, mh_idx.reshape(B, N_MH * HIST)], axis=1
    ).astype(np.int32)  # (B, 88)

    def pack_k(Wmat, out_cols):
        # (448, out_cols) -> (128, 4, out_cols) k-tiles, zero padded
        p = np.zeros((128, 4, out_cols), np.float32)
        for j in range(4):
            p[0 : KS[j], j, :] = Wmat[j * 128 : j * 128 + KS[j], :]
        return p.astype(BF16)

    w1p = pack_k(np.asarray(W1, np.float32), 1024)
    w2p = (
        np.asarray(W2, np.float32)
        .reshape(8, 128, 512)
        .transpose(1, 0, 2)
        .copy()
        .astype(BF16)
    )
    w3p = (
        np.asarray(W3, np.float32)
        .reshape(4, 128, 256)
        .transpose(1, 0, 2)
        .copy()
        .astype(BF16)
    )
    lw = np.asarray(lin_w, np.float32)[:, 0]
    cwq = pack_k(
        np.concatenate([np.asarray(cross_w, np.float32).T, lw[:IN_DIM, None]], 1), 5
    )  # (128, 4, 5) bf16
    wsm = np.zeros((128, 22), np.float32)
    wsm[:, 0:20] = cwq.astype(np.float32).reshape(128, 20)
    wsm[:, 20:22] = lw[IN_DIM:].reshape(2, 128).T
    wsm = wsm.astype(BF16)
    biasp = np.concatenate(
        [
            np.asarray(b1, np.float32).reshape(8, 128).T,
            np.asarray(b2, np.float32).reshape(4, 128).T,
            np.asarray(b3, np.float32).reshape(2, 128).T,
        ],
        axis=1,
    ).copy()

    # cross-net constants: C_0 = 0, C_{l+1} = C_l + b_l ; c_l = C_l . w_l
    cb = np.asarray(cross_b, np.float64)
    cwf = np.asarray(cross_w, np.float64)
    C = np.zeros(IN_DIM, np.float64)
    c_consts = []
    for l in range(4):
        c_consts.append(float(C @ cwf[l]))
        C = C + cb[l]
    sig_bias = float(C @ np.asarray(lw[:IN_DIM], np.float64)) + float(
        np.asarray(lin_b, np.float64).reshape(-1)[0]
    )
    if any(abs(c) > 1e-30 for c in c_consts):
        raise NotImplementedError(
            "cross_b != 0 would need the general recurrence; this model's "
            "setup always has cross_b == 0"
        )

    shared = {
        "ohtab": oh_tab,
        "mhtab": mh_tab,
        "w1p": w1p,
        "w2p": w2p,
        "w3p": w3p,
        "wsm": wsm,
        "biasp": biasp,
    }
    in_maps = []
    for core in range(N_CORES):
        rs = slice(core * BL, (core + 1) * BL)
        # chunk-major -> partition-major [128, n_chunks, :] staging layout
        m = dict(shared)
        m["dense"] = np.ascontiguousarray(
            dense_bf[rs].reshape(BL // 128, 128, DENSE).transpose(1, 0, 2)
        )
        m["idx"] = np.ascontiguousarray(
            idx_all[rs].reshape(BL // 128, 128, 88).transpose(1, 0, 2)
        )
        in_maps.append(m)
    return in_maps, c_consts, sig_bias


def _run(inputs, trace=False, debug=False):
    from concourse.bass_utils import run_bass_kernel_spmd

    in_maps, c_consts, sig_bias = _prep_inputs(**inputs)
    nc = _build_program(c_consts, sig_bias, debug=debug)
    res = run_bass_kernel_spmd(
        nc, in_maps, core_ids=list(range(N_CORES)), trace=trace
    )
    outs = [
        res.results[c]["out"].reshape(128, BL // 128).T.reshape(BL)
        for c in range(N_CORES)
    ]
    full = np.concatenate(outs).reshape(B, 1).astype(np.float32)
    return full, res


def kernel(**inputs):
    full, _ = _run(inputs, trace=False)
    return full



# revision 6
# speedup vs baseline: 2.3856x; 2.3856x over previous
"""DCN-v1 (dense_mlp) Trainium2 kernel.

Strategy (8 NeuronCores, SPMD):
  - Data-parallel over batch: 16384 rows -> 2048 per core.
  - Embedding tables replicated per core (bf16, flattened per-field with
    host-side index offsetting); lookups via indirect DMA gathers.
  - Multi-hot sum-pool via one strided DVE reduction per 128-row chunk.
  - Activations kept feature-major (x^T) on chip; weights are the matmul
    stationary operand, batch streams as the moving operand.
  - CrossNet collapsed algebraically: x_i = A_i * x0 + C_i with A_i a
    per-sample scalar and C_i an input-derived constant vector, so the whole
    cross stack + final-layer cross dot reduce to ONE matmul
    P = x0 @ [w_0..w_3, lin_w[:448]] plus a tiny scalar recurrence.
  - MLP in bf16 with fp32 PSUM accumulation; ReLU+bias fused on ScalarE.

Performance notes (measured, 8 cores):
  - HW exec ~2.04 ms, rel err (l2) 1.15e-3 vs the fp32 reference.
  - The time is 1408 indirect-DMA gathers x ~1.45 us of serialized SWDGE
    descriptor generation; every other engine (PE ~75 us/core, DVE, ACT)
    and the transferred bytes hide completely under it.
  - The gather count is the floor for this primitive: indirect DMA moves
    exactly one table row per partition per instruction (multi-index offset
    APs silently collapse to idx[p,0] on HW, verified empirically).
  - dma_gather (InstDMAGatherAnt) would cut this ~6x via 4-row-grouped
    int16 super-indices + mask-fused sub-row select (see kernel2.py), but
    it does not execute under the current PJRT/axon runtime (leaves the
    exec unit unrecoverable; reproduced with a minimal standalone kernel).
"""

import os
import sys

import numpy as np
import ml_dtypes

for _p in ("/opt/trn_rl_repo", os.path.expanduser("~/.axon_site/_ro/trn_rl_repo")):
    if os.path.isdir(_p) and _p not in sys.path:
        sys.path.append(_p)

B = 16384
N_CORES = 8
BL = B // N_CORES  # 2048 rows per core
DENSE = 64
N_OH, N_MH, HIST = 8, 4, 20
VOCAB = 100000
EMB = 32
IN_DIM = 448
HID = [1024, 512, 256]
CHUNK = 128  # samples per gather/transpose chunk
NBLK = 512  # samples per matmul n-block
KS = [128, 128, 128, 64]  # k-tile sizes over the 448-dim input features
NGRP = VOCAB // 4  # 25000 4-row groups per mh field table
MH_G = [(0, 8), (8, 16), (16, 20)]  # hist ranges of the 3 dma_gathers/(c,f)
BF16 = ml_dtypes.bfloat16


def _build_program(c_consts, sig_bias, debug=False):
    """Build the SPMD Bass/Tile program. c_consts[l] = C_l . w_l (fp32 floats),
    sig_bias = C_4 . lin_w[:448] + lin_b."""
    from contextlib import ExitStack

    import concourse.bass as bass
    import concourse.tile as tile
    from concourse import bacc, mybir
    from concourse.masks import make_identity

    dt = mybir.dt
    AF = mybir.ActivationFunctionType
    n_chunks_per_nb = NBLK // CHUNK  # 4
    n_nb = BL // NBLK  # 4

    nc = bacc.Bacc(num_swdge_queues=4)
    dense_d = nc.dram_tensor(
        "dense", [128, BL // 128, DENSE], dt.bfloat16, kind="ExternalInput"
    )
    idx_d = nc.dram_tensor("idx", [128, BL // 128, 8], dt.int32, kind="ExternalInput")
    ohtab_d = nc.dram_tensor(
        "ohtab", [N_OH * VOCAB, EMB], dt.bfloat16, kind="ExternalInput"
    )
    mhtab_d = nc.dram_tensor(
        "mhtab", [N_MH * NGRP, 4 * EMB], dt.bfloat16, kind="ExternalInput"
    )
    mhidx_d = nc.dram_tensor(
        "mhidx", [128, BL // 128, N_MH, 160], dt.int16, kind="ExternalInput"
    )
    msk_d = nc.dram_tensor(
        "msk", [128, BL // 128, N_MH, 4, HIST], dt.bfloat16, kind="ExternalInput"
    )
    w1_d = nc.dram_tensor("w1p", [128, 4, 1024], dt.bfloat16, kind="ExternalInput")
    w2_d = nc.dram_tensor("w2p", [128, 8, 512], dt.bfloat16, kind="ExternalInput")
    w3_d = nc.dram_tensor("w3p", [128, 4, 256], dt.bfloat16, kind="ExternalInput")
    wsm_d = nc.dram_tensor("wsm", [128, 22], dt.bfloat16, kind="ExternalInput")
    bias_d = nc.dram_tensor("biasp", [128, 14], dt.float32, kind="ExternalInput")
    out_d = nc.dram_tensor("out", [128, BL // 128], dt.float32, kind="ExternalOutput")
    if debug:
        dbgx_d = nc.dram_tensor(
            "dbg_x0", [128, BL // 128, 512], dt.bfloat16, kind="ExternalOutput"
        )
        dbgp_d = nc.dram_tensor(
            "dbg_pn", [128, BL // 128, 5], dt.float32, kind="ExternalOutput"
        )

    with ExitStack() as ctx:
        tc = ctx.enter_context(tile.TileContext(nc))
        wp = ctx.enter_context(tc.tile_pool(name="weights", bufs=1))
        # DMA-written tiles get one slot per chunk: the DIRECT2D pseudo-DMA
        # ISA struct only has a single sync-wait slot, so gathers must not
        # carry WAR/WAW waits from slot reuse.
        x0p = ctx.enter_context(tc.tile_pool(name="x0", bufs=3))
        gtp = ctx.enter_context(tc.tile_pool(name="gt", bufs=16))
        mhp = ctx.enter_context(tc.tile_pool(name="mh", bufs=2))
        # per-queue dst pools: bufs=2 bounds in-flight descriptors per SWDGE
        # queue (ring capacity ~8K descs; 3 x 2048 in flight is the safe max)
        dgp = [
            ctx.enter_context(tc.tile_pool(name=f"dg{q}", bufs=2)) for q in range(4)
        ]
        selp = ctx.enter_context(tc.tile_pool(name="sel", bufs=2))
        xtp = ctx.enter_context(tc.tile_pool(name="xt", bufs=2))
        hp = ctx.enter_context(tc.tile_pool(name="h", bufs=2))
        recp = ctx.enter_context(tc.tile_pool(name="rec", bufs=2))
        ps_mm = ctx.enter_context(tc.tile_pool(name="psmm", bufs=3, space="PSUM"))
        ps_tr = ctx.enter_context(tc.tile_pool(name="pstr", bufs=2, space="PSUM"))
        ps_sm = ctx.enter_context(tc.tile_pool(name="pssm", bufs=1, space="PSUM"))
        ps_q2 = ctx.enter_context(tc.tile_pool(name="psq2", bufs=2, space="PSUM"))

        # --- resident weights ---
        w1_sb = wp.tile([128, 4, 1024], dt.bfloat16)
        nc.sync.dma_start(w1_sb[:], w1_d[:])
        w2_sb = wp.tile([128, 8, 512], dt.bfloat16)
        nc.sync.dma_start(w2_sb[:], w2_d[:])
        w3_sb = wp.tile([128, 4, 256], dt.bfloat16)
        nc.sync.dma_start(w3_sb[:], w3_d[:])
        wsm_sb = wp.tile([128, 22], dt.bfloat16)
        nc.sync.dma_start(wsm_sb[:], wsm_d[:])
        bias_sb = wp.tile([128, 14], dt.float32)
        nc.sync.dma_start(bias_sb[:], bias_d[:])
        ident = wp.tile([128, 128], dt.bfloat16)
        make_identity(nc, ident[:])
        # whole-core index + dense staging: one DMA each, resident in SBUF,
        # so per-chunk gathers carry no DMA-RAW waits (ISA wait-slot limits)
        idx_sb = wp.tile([128, BL // 128, 8], dt.int32)
        nc.sync.dma_start(idx_sb[:], idx_d[:])
        mhidx_sb = wp.tile([128, BL // 128, N_MH, 160], dt.int16)
        nc.sync.dma_start(mhidx_sb[:], mhidx_d[:])
        msk_sb = wp.tile([128, BL // 128, N_MH, 4, HIST], dt.bfloat16)
        nc.sync.dma_start(msk_sb[:], msk_d[:])
        dense_sb = wp.tile([128, BL // 128, DENSE], dt.bfloat16)
        nc.sync.dma_start(dense_sb[:], dense_d[:])
        out_sb = wp.tile([128, BL // 128], dt.float32)

        for nb in range(n_nb):
            # ---- build x0^T [feat, 512] for this n-block, 128 samples at a time
            x0T = xtp.tile([128, 4, NBLK], dt.bfloat16, tag="x0T")
            lgq1 = recp.tile([128, 4], dt.float32, tag="lgq1")
            for cc in range(n_chunks_per_nb):
                c = nb * n_chunks_per_nb + cc
                rs = slice(c * CHUNK, (c + 1) * CHUNK)
                cs = slice(cc * CHUNK, (cc + 1) * CHUNK)

                # indirect DMA moves one table row per partition per
                # instruction (multi-index offset APs silently collapse to
                # idx[p,0] on HW), so each of the 88 lookup slots is its own
                # gather instruction.
                x0n = gtp.tile([128, 384], dt.bfloat16, tag="x0n")
                for k in range(N_OH):
                    nc.gpsimd.indirect_dma_start(
                        out=x0n[:, k * EMB : (k + 1) * EMB],
                        out_offset=None,
                        in_=ohtab_d[:, :],
                        in_offset=bass.IndirectOffsetOnAxis(
                            ap=idx_sb[:, c, k : k + 1], axis=0
                        ),
                    )
                # multi-hot via dma_gather (mlp ucode): per field, 3 gathers
                # of 4-row 256B groups on rotating SWDGE queues, then a
                # mask-fused sub-row select + hist-sum on DVE.
                mh_ps = mhp.tile([128, N_MH * EMB], dt.float32, tag="mhpool")
                for f in range(N_MH):
                    dsts = []
                    for t, (h0, h1) in enumerate(MH_G):
                        nh = h1 - h0
                        q = (((c * N_MH) + f) * 3 + t) % 4
                        dg = dgp[q].tile(
                            [128, nh, 4 * EMB], dt.bfloat16, tag=f"dg{nh}", name=f"dg{q}_{nh}"
                        )
                        nc.gpsimd.dma_gather(
                            dg[:],
                            mhtab_d[f * NGRP : (f + 1) * NGRP, :],
                            mhidx_sb[:, c, f, 8 * h0 : 8 * h1],
                            nh * 128,
                            nh * 128,
                            4 * EMB,
                            queue_num=q,
                        )
                        dsts.append(dg)
                    sel = selp.tile([128, 4, HIST, EMB], dt.bfloat16, tag="sel")
                    for sub in range(4):
                        for t, (h0, h1) in enumerate(MH_G):
                            nc.vector.tensor_mul(
                                sel[:, sub, h0:h1, :],
                                dsts[t][:, :, sub * EMB : (sub + 1) * EMB],
                                msk_sb[:, c, f, sub, h0:h1]
                                .unsqueeze(2)
                                .broadcast_to([128, h1 - h0, EMB]),
                            )
                    nc.vector.tensor_reduce(
                        out=mh_ps[:, f * EMB : (f + 1) * EMB],
                        in_=sel[:].rearrange("p s h e -> p e (s h)"),
                        axis=mybir.AxisListType.X,
                        op=mybir.AluOpType.add,
                    )
                nc.vector.tensor_copy(x0n[:, 256:384], mh_ps[:])

                # consolidate to a single-engine-writer tile: LDWEIGHTS (the
                # transpose reads x0 as the stationary operand) only supports
                # one sync wait, but the pieces come from several engines.
                x0c = x0p.tile([128, 512], dt.bfloat16, tag="x0c")
                nc.vector.memset(x0c[:, 448:512], 0.0)
                nc.vector.tensor_copy(x0c[:, 0:DENSE], dense_sb[:, c, :])
                nc.vector.tensor_copy(x0c[:, DENSE:448], x0n[:])

                # transpose the 128-sample chunk to feature-major
                tp = ps_tr.tile([128, 4, 128], dt.bfloat16, tag="trps")
                for j in range(4):
                    nc.tensor.transpose(
                        tp[:, j : j + 1, :],
                        x0c[:, j * 128 : (j + 1) * 128],
                        ident[:],
                    )
                nc.vector.tensor_copy(x0T[:, :, cs], tp[:])

                # cross projections for this chunk, sample-major:
                # pn[s, l] = x0 . w_l (l<4), pn[s, 4] = x0 . lin_w[:448]
                pn = ps_sm.tile([128, 5], dt.float32, tag="pn")
                for j in range(4):
                    nc.tensor.matmul(
                        pn[:],
                        x0T[0 : KS[j], j : j + 1, cs],
                        wsm_sb[0 : KS[j], j * 5 : j * 5 + 5],
                        start=(j == 0),
                        stop=(j == 3),
                    )
                # logit cross part: prod(1+p_l) * q1  (cross_b == 0)
                if debug:
                    nc.sync.dma_start(dbgx_d[:, c, :], x0c[:])
                    dbgp = recp.tile([128, 5], dt.float32, tag="dbgp")
                    nc.vector.tensor_copy(dbgp[:], pn[:])
                    nc.sync.dma_start(dbgp_d[:, c, :], dbgp[:])
                pp1 = recp.tile([128, 4], dt.float32, tag="pp1")
                nc.vector.tensor_scalar_add(pp1[:], pn[:, 0:4], 1.0)
                m01 = recp.tile([128, 1], dt.float32, tag="m01")
                nc.vector.tensor_mul(m01[:], pp1[:, 0:1], pp1[:, 1:2])
                m23 = recp.tile([128, 1], dt.float32, tag="m23")
                nc.vector.tensor_mul(m23[:], pp1[:, 2:3], pp1[:, 3:4])
                a4 = recp.tile([128, 1], dt.float32, tag="a4")
                nc.vector.tensor_mul(a4[:], m01[:], m23[:])
                nc.vector.tensor_mul(lgq1[:, cc : cc + 1], a4[:], pn[:, 4:5])

            # ---- deep net ----
            h1 = hp.tile([128, 8, NBLK], dt.bfloat16, tag="h1")
            for m in range(8):
                ps = ps_mm.tile([128, NBLK], dt.float32, tag="mm")
                for j in range(4):
                    nc.tensor.matmul(
                        ps[:],
                        w1_sb[0 : KS[j], j : j + 1, m * 128 : (m + 1) * 128],
                        x0T[0 : KS[j], j : j + 1, :],
                        start=(j == 0),
                        stop=(j == 3),
                    )
                nc.scalar.activation(
                    h1[:, m : m + 1, :], ps[:], AF.Relu, bias=bias_sb[:, m : m + 1]
                )
            h2 = hp.tile([128, 4, NBLK], dt.bfloat16, tag="h2")
            for m in range(4):
                ps = ps_mm.tile([128, NBLK], dt.float32, tag="mm")
                for j in range(8):
                    nc.tensor.matmul(
                        ps[:],
                        w2_sb[:, j : j + 1, m * 128 : (m + 1) * 128],
                        h1[:, j : j + 1, :],
                        start=(j == 0),
                        stop=(j == 7),
                    )
                nc.scalar.activation(
                    h2[:, m : m + 1, :], ps[:], AF.Relu, bias=bias_sb[:, 8 + m : 9 + m]
                )
            h3 = hp.tile([128, 2, NBLK], dt.bfloat16, tag="h3")
            for m in range(2):
                ps = ps_mm.tile([128, NBLK], dt.float32, tag="mm")
                for j in range(4):
                    nc.tensor.matmul(
                        ps[:],
                        w3_sb[:, j : j + 1, m * 128 : (m + 1) * 128],
                        h2[:, j : j + 1, :],
                        start=(j == 0),
                        stop=(j == 3),
                    )
                nc.scalar.activation(
                    h3[:, m : m + 1, :], ps[:], AF.Relu, bias=bias_sb[:, 12 + m : 13 + m]
                )

            # ---- final: logit = prod(1+p)*q1 + h3.lin_w_bot + sig_bias ----
            for cc in range(n_chunks_per_nb):
                c = nb * n_chunks_per_nb + cc
                cs = slice(cc * CHUNK, (cc + 1) * CHUNK)
                q2n = ps_q2.tile([128, 1], dt.float32, tag="q2n")
                for j in range(2):
                    nc.tensor.matmul(
                        q2n[:],
                        h3[:, j : j + 1, cs],
                        wsm_sb[:, 20 + j : 21 + j],
                        start=(j == 0),
                        stop=(j == 1),
                    )
                lg2 = recp.tile([128, 1], dt.float32, tag="lg2")
                nc.vector.tensor_add(lg2[:], lgq1[:, cc : cc + 1], q2n[:])
                nc.scalar.activation(
                    out_sb[:, c : c + 1], lg2[:], AF.Sigmoid, bias=float(sig_bias)
                )

        nc.sync.dma_start(out_d[:], out_sb[:])

    nc.compile()
    return nc


def _prep_inputs(
    dense_x,
    one_hot_x,
    multi_hot_x,
    one_hot_emb,
    multi_hot_emb,
    cross_w,
    cross_b,
    W1,
    b1,
    W2,
    b2,
    W3,
    b3,
    lin_w,
    lin_b,
):
    dense_bf = np.ascontiguousarray(dense_x, dtype=np.float32).astype(BF16)
    oh_tab = np.ascontiguousarray(
        one_hot_emb.reshape(N_OH * VOCAB, EMB), dtype=np.float32
    ).astype(BF16)
    mh_tab = np.ascontiguousarray(
        multi_hot_emb.reshape(N_MH * NGRP, 4 * EMB), dtype=np.float32
    ).astype(BF16)

    oh_idx = one_hot_x.astype(np.int64) + (np.arange(N_OH, dtype=np.int64) * VOCAB)
    idx_all = oh_idx.astype(np.int32)  # (B, 8)

    mh = np.asarray(multi_hot_x).astype(np.int64)  # (B, N_MH, HIST)
    mh_grp = (mh >> 2).astype(np.int16)  # local 4-row group per field
    mh_sub = (mh & 3).astype(np.int64)
    # masks (B, N_MH, 4, HIST) bf16: one-hot of sub per (sample, field, hist)
    msk_all = (
        (mh_sub[:, :, None, :] == np.arange(4).reshape(1, 1, 4, 1))
        .astype(np.float32)
        .astype(BF16)
    )

    def wrap_gidx(arr):
        # arr (128, nh): lookup (s, h) at flat pos h*128+s; wrap into 16
        # partitions (pos -> partition pos%16, col pos//16), replicate x8
        flat = arr.T.reshape(-1)  # h-major
        w16 = flat.reshape(-1, 16).T  # (16, nh*8)
        return np.tile(w16, (8, 1))  # (128, nh*8)

    def pack_k(Wmat, out_cols):
        # (448, out_cols) -> (128, 4, out_cols) k-tiles, zero padded
        p = np.zeros((128, 4, out_cols), np.float32)
        for j in range(4):
            p[0 : KS[j], j, :] = Wmat[j * 128 : j * 128 + KS[j], :]
        return p.astype(BF16)

    w1p = pack_k(np.asarray(W1, np.float32), 1024)
    w2p = (
        np.asarray(W2, np.float32)
        .reshape(8, 128, 512)
        .transpose(1, 0, 2)
        .copy()
        .astype(BF16)
    )
    w3p = (
        np.asarray(W3, np.float32)
        .reshape(4, 128, 256)
        .transpose(1, 0, 2)
        .copy()
        .astype(BF16)
    )
    lw = np.asarray(lin_w, np.float32)[:, 0]
    cwq = pack_k(
        np.concatenate([np.asarray(cross_w, np.float32).T, lw[:IN_DIM, None]], 1), 5
    )  # (128, 4, 5) bf16
    wsm = np.zeros((128, 22), np.float32)
    wsm[:, 0:20] = cwq.astype(np.float32).reshape(128, 20)
    wsm[:, 20:22] = lw[IN_DIM:].reshape(2, 128).T
    wsm = wsm.astype(BF16)
    biasp = np.concatenate(
        [
            np.asarray(b1, np.float32).reshape(8, 128).T,
            np.asarray(b2, np.float32).reshape(4, 128).T,
            np.asarray(b3, np.float32).reshape(2, 128).T,
        ],
        axis=1,
    ).copy()

    # cross-net constants: C_0 = 0, C_{l+1} = C_l + b_l ; c_l = C_l . w_l
    cb = np.asarray(cross_b, np.float64)
    cwf = np.asarray(cross_w, np.float64)
    C = np.zeros(IN_DIM, np.float64)
    c_consts = []
    for l in range(4):
        c_consts.append(float(C @ cwf[l]))
        C = C + cb[l]
    sig_bias = float(C @ np.asarray(lw[:IN_DIM], np.float64)) + float(
        np.asarray(lin_b, np.float64).reshape(-1)[0]
    )
    if any(abs(c) > 1e-30 for c in c_consts):
        raise NotImplementedError(
            "cross_b != 0 would need the general recurrence; this model's "
            "setup always has cross_b == 0"
        )

    shared = {
        "ohtab": oh_tab,
        "mhtab": mh_tab,
        "w1p": w1p,
        "w2p": w2p,
        "w3p": w3p,
        "wsm": wsm,
        "biasp": biasp,
    }
    in_maps = []
    for core in range(N_CORES):
        rs = slice(core * BL, (core + 1) * BL)
        # chunk-major -> partition-major [128, n_chunks, :] staging layout
        m = dict(shared)
        m["dense"] = np.ascontiguousarray(
            dense_bf[rs].reshape(BL // 128, 128, DENSE).transpose(1, 0, 2)
        )
        m["idx"] = np.ascontiguousarray(
            idx_all[rs].reshape(BL // 128, 128, 8).transpose(1, 0, 2)
        )
        nck = BL // 128
        grp_c = mh_grp[rs].reshape(nck, 128, N_MH, HIST)
        mi = np.zeros((128, nck, N_MH, 160), np.int16)
        for c in range(nck):
            for f in range(N_MH):
                col = 0
                for h0, h1 in ((0, 8), (8, 16), (16, 20)):
                    arr = grp_c[c, :, f, h0:h1]  # (128, nh)
                    w = wrap_gidx(arr)  # (128, nh*8)
                    mi[:, c, f, col : col + w.shape[1]] = w
                    col += w.shape[1]
        m["mhidx"] = mi
        m["msk"] = np.ascontiguousarray(
            msk_all[rs]
            .reshape(nck, 128, N_MH, 4, HIST)
            .transpose(1, 0, 2, 3, 4)
        )
        in_maps.append(m)
    return in_maps, c_consts, sig_bias


def _run(inputs, trace=False, debug=False):
    from concourse.bass_utils import run_bass_kernel_spmd

    in_maps, c_consts, sig_bias = _prep_inputs(**inputs)
    nc = _build_program(c_consts, sig_bias, debug=debug)
    res = run_bass_kernel_spmd(
        nc, in_maps, core_ids=list(range(N_CORES)), trace=trace
    )
    outs = [
        res.results[c]["out"].reshape(128, BL // 128).T.reshape(BL)
        for c in range(N_CORES)
    ]
    full = np.concatenate(outs).reshape(B, 1).astype(np.float32)
    return full, res


def kernel(**inputs):
    full, _ = _run(inputs, trace=False)
    return full

